# revision 14
# baseline (speedup 1.0000x reference)
"""DeepSeek-style MoE block (grouped top-k routing + 16 routed experts +
shared expert) on 8 Trainium2 NeuronCores.

Sharding: expert-parallel. Core c owns routed experts {2c, 2c+1} (dense
all-token compute, weighted by the combine matrix) plus a 1/8 slice of the
shared expert intermediate dim. Each core computes the full router from a
column-permuted gate matrix so its own experts always land in combine rows
0/1 (keeps the program core-independent). Each core emits an fp32 partial
output [H, T]; partials are summed and transposed on the host.

Math notes:
 - softmax + renormalized top-k weights: the softmax denominator cancels in
   the renormalization, so selection + weights use exp(logit - max) only.
 - ROUTED_SCALING is folded into the combine weights.

All activations/weights are pre-transposed/tiled on the host so every
matmul consumes contiguous [128, 128] weight blocks (stationary) and
[128, T] activation slabs (moving).
"""

import sys

sys.path.insert(0, "/opt/trn_rl_repo")

from contextlib import ExitStack

import numpy as np
import ml_dtypes

import concourse.bass as bass
import concourse.mybir as mybir
from concourse import bacc
from concourse.bass import ts
from concourse.tile import TileContext
from concourse.bass_utils import run_bass_kernel_spmd
from concourse.masks import make_identity

F32 = mybir.dt.float32

T, H, E, I = 1024, 2048, 16, 704
IS = 2 * I  # shared expert intermediate
TOP_K, N_GROUP, TOPK_GROUP = 6, 4, 2
ROUTED_SCALING = 2.5

N_CORES = 8
EPC = E // N_CORES  # experts per core (2)
SHI = IS // N_CORES  # shared intermediate slice per core (176)
KB = H // 128  # 16 contraction blocks over hidden dim
GJ = (I + 127) // 128  # 6 col-pair blocks per routed expert
SJ = (SHI + 127) // 128  # 2 col-pair blocks for shared slice
NH = 2  # matmul N split (512 tokens per matmul)
NS = T // NH
MB = H // 128  # 16 output row blocks


def _pad_cols(a, n):
    # a: [rows, c] -> [rows, n] zero-padded
    out = np.zeros((a.shape[0], n), a.dtype)
    out[:, : a.shape[1]] = a
    return out


def _pad_rows(a, n):
    out = np.zeros((n, a.shape[1]), a.dtype)
    out[: a.shape[0], :] = a
    return out


def _expert_perm(c):
    """Permute experts so core c's experts (2c, 2c+1) map to rows 0, 1 while
    preserving the 4-expert group-block structure (group order and
    within-group order are both free)."""
    g = c // 2
    r = (c % 2) * 2
    within = [r, r + 1] + [x for x in range(4) if x not in (r, r + 1)]
    groups = [g] + [x for x in range(N_GROUP) if x != g]
    return [4 * gg + w for gg in groups for w in within]


def _prep_core(c, hs, w_gate, w_gate_up, w_down, w_sgu, w_sd, np_lo):
    f32 = np.float32
    xt = np.ascontiguousarray(hs.T).astype(f32)  # [H, T]
    ins = {"xt32": xt}
    if np_lo != f32:
        ins["xt_lo"] = xt.astype(np_lo)

    perm = _expert_perm(c)
    wg = np.ascontiguousarray(w_gate[:, perm]).astype(f32)  # [H, E]
    ins["wg"] = np.ascontiguousarray(wg.reshape(KB, 128, E))

    e0 = 2 * c
    wgu = w_gate_up[e0 : e0 + EPC].astype(np_lo)  # [2, H, 2I]
    wdn = w_down[e0 : e0 + EPC].astype(np_lo)  # [2, I, H]

    wgu_g = np.zeros((EPC, GJ, KB, 128, 128), np_lo)
    wgu_u = np.zeros((EPC, GJ, KB, 128, 128), np_lo)
    wd_t = np.zeros((EPC, MB, GJ, 128, 128), np_lo)
    for e in range(EPC):
        for j in range(GJ):
            w = min(128, I - 128 * j)
            for k in range(KB):
                wgu_g[e, j, k, :, :w] = wgu[e, 128 * k : 128 * (k + 1),
                                            128 * j : 128 * j + w]
                wgu_u[e, j, k, :, :w] = wgu[e, 128 * k : 128 * (k + 1),
                                            I + 128 * j : I + 128 * j + w]
            for m in range(MB):
                wd_t[e, m, j, :w, :] = wdn[e, 128 * j : 128 * j + w,
                                           128 * m : 128 * (m + 1)]
    ins["wgu_g"], ins["wgu_u"], ins["wd"] = wgu_g, wgu_u, wd_t

    # shared expert slice: intermediate rows [c*SHI, (c+1)*SHI)
    s0 = c * SHI
    sg = w_sgu[:, s0 : s0 + SHI].astype(np_lo)  # [H, SHI]
    su = w_sgu[:, IS + s0 : IS + s0 + SHI].astype(np_lo)
    sd = w_sd[s0 : s0 + SHI, :].astype(np_lo)  # [SHI, H]

    wsg_g = np.zeros((SJ, KB, 128, 128), np_lo)
    wsg_u = np.zeros((SJ, KB, 128, 128), np_lo)
    wsd_t = np.zeros((MB, SJ, 128, 128), np_lo)
    for j in range(SJ):
        w = min(128, SHI - 128 * j)
        for k in range(KB):
            wsg_g[j, k, :, :w] = sg[128 * k : 128 * (k + 1),
                                    128 * j : 128 * j + w]
            wsg_u[j, k, :, :w] = su[128 * k : 128 * (k + 1),
                                    128 * j : 128 * j + w]
        for m in range(MB):
            wsd_t[m, j, :w, :] = sd[128 * j : 128 * j + w,
                                    128 * m : 128 * (m + 1)]
    ins["wsg_g"], ins["wsg_u"], ins["wsd"] = wsg_g, wsg_u, wsd_t
    return ins


def build(low=F32):
    nc = bacc.Bacc("TRN2", target_bir_lowering=False, debug=False,
                   num_devices=N_CORES)
    A = mybir.AluOpType
    X = mybir.AxisListType.X
    AF = mybir.ActivationFunctionType

    xt32_d = nc.dram_tensor("xt32", [H, T], F32, kind="ExternalInput")
    xlo_d = (xt32_d if low == F32 else
             nc.dram_tensor("xt_lo", [H, T], low, kind="ExternalInput"))
    wg_d = nc.dram_tensor("wg", [KB, 128, E], F32, kind="ExternalInput")
    wgu_g_d = nc.dram_tensor("wgu_g", [EPC, GJ, KB, 128, 128], low,
                             kind="ExternalInput")
    wgu_u_d = nc.dram_tensor("wgu_u", [EPC, GJ, KB, 128, 128], low,
                             kind="ExternalInput")
    wd_d = nc.dram_tensor("wd", [EPC, MB, GJ, 128, 128], low,
                          kind="ExternalInput")
    wsg_g_d = nc.dram_tensor("wsg_g", [SJ, KB, 128, 128], low,
                             kind="ExternalInput")
    wsg_u_d = nc.dram_tensor("wsg_u", [SJ, KB, 128, 128], low,
                             kind="ExternalInput")
    wsd_d = nc.dram_tensor("wsd", [MB, SJ, 128, 128], low,
                           kind="ExternalInput")
    part_d = nc.dram_tensor("part", [H, T], F32, kind="ExternalOutput")

    with TileContext(nc) as tc, ExitStack() as ctx:
        ep = ctx.enter_context  # shorthand

        # ---- resident SBUF ----
        xtp = ep(tc.tile_pool(name="xt32p", bufs=KB))
        xt32 = [xtp.tile([128, T], F32, tag="xt32", name=f"xt32_{k}")
                for k in range(KB)]
        for k in range(KB):
            nc.sync.dma_start(out=xt32[k][:, :], in_=xt32_d[ts(k, 128), :])
        if low == F32:
            xlo = xt32
        else:
            xlp = ep(tc.tile_pool(name="xlop", bufs=KB))
            xlo = [xlp.tile([128, T], low, tag="xlo", name=f"xlo_{k}")
                   for k in range(KB)]
            for k in range(KB):
                nc.sync.dma_start(out=xlo[k][:, :], in_=xlo_d[ts(k, 128), :])

        cstp = ep(tc.tile_pool(name="cstp", bufs=1))
        wgsb = cstp.tile([128, KB * E], F32, tag="wgsb")
        for k in range(KB):
            nc.sync.dma_start(out=wgsb[:, ts(k, E)], in_=wg_d[k, :, :])
        ident = cstp.tile([128, 128], F32, tag="ident")
        make_identity(nc, ident[:, :])

        actp = ep(tc.tile_pool(name="actp", bufs=EPC * GJ + SJ))
        act = [[actp.tile([128, T], low, tag="act", name=f"act_{e}_{j}")
                for j in range(GJ)] for e in range(EPC)]
        acts = [actp.tile([128, T], low, tag="act", name=f"acts_{j}")
                for j in range(SJ)]
        bcp = ep(tc.tile_pool(name="bcp", bufs=EPC))
        bc = [bcp.tile([128, T], F32, tag="bc", name=f"bc_{e}")
              for e in range(EPC)]

        # ---- phase 1: router ----
        with tc.tile_pool(name="rt_ps", bufs=2, space="PSUM") as rtp, \
             tc.tile_pool(name="ct_ps", bufs=EPC, space="PSUM") as ctp, \
             tc.tile_pool(name="rsm", bufs=3) as rsm, \
             tc.tile_pool(name="rwk", bufs=3) as rwk:
            ct = [ctp.tile([1, T], F32, tag="ct", name=f"ct_{e}")
                  for e in range(EPC)]
            for t in range(T // 128):
                pl = rtp.tile([128, E], F32, tag="pl")
                for k in range(KB):
                    nc.tensor.matmul(pl[:, :], lhsT=xt32[k][:, ts(t, 128)],
                                     rhs=wgsb[:, ts(k, E)],
                                     start=(k == 0), stop=(k == KB - 1))
                nm = rsm.tile([128, 1], F32, tag="nm")
                nc.vector.tensor_reduce(nm[:, :], pl[:, :], X, A.max,
                                        negate=True)
                es = rsm.tile([128, E], F32, tag="es")
                nc.scalar.activation(es[:, :], pl[:, :], AF.Exp, bias=nm[:, :])
                gmax = rsm.tile([128, N_GROUP], F32, tag="gmax")
                nc.vector.tensor_reduce(
                    gmax[:, :],
                    es[:, :].rearrange("p (g e) -> p g e", g=N_GROUP),
                    X, A.max)
                m1 = rsm.tile([128, 1], F32, tag="m1")
                nc.vector.tensor_reduce(m1[:, :], gmax[:, :], X, A.max)
                gz = rsm.tile([128, N_GROUP], F32, tag="gz")
                nc.vector.scalar_tensor_tensor(
                    out=gz[:, :], in0=gmax[:, :], scalar=m1[:, :],
                    in1=gmax[:, :], op0=A.is_lt, op1=A.mult)
                m2 = rsm.tile([128, 1], F32, tag="m2")
                nc.vector.tensor_reduce(m2[:, :], gz[:, :], X, A.max)
                keep = rsm.tile([128, N_GROUP], F32, tag="keep")
                nc.vector.tensor_scalar(
                    out=keep[:, :], in0=gmax[:, :], scalar1=m2[:, :],
                    scalar2=None, op0=A.is_ge)
                msk = rsm.tile([128, E], F32, tag="msk")
                for g in range(N_GROUP):
                    nc.vector.tensor_scalar(
                        out=msk[:, 4 * g : 4 * g + 4],
                        in0=es[:, 4 * g : 4 * g + 4],
                        scalar1=keep[:, g : g + 1], scalar2=None, op0=A.mult)
                mxs = rsm.tile([128, TOP_K], F32, tag="mxs")
                wcur = msk
                for i in range(TOP_K):
                    nc.vector.tensor_reduce(mxs[:, i : i + 1], wcur[:, :],
                                            X, A.max)
                    wnxt = rwk.tile([128, E], F32, tag="wk")
                    nc.vector.scalar_tensor_tensor(
                        out=wnxt[:, :], in0=wcur[:, :],
                        scalar=mxs[:, i : i + 1], in1=wcur[:, :],
                        op0=A.is_lt, op1=A.mult)
                    wcur = wnxt
                wsum = rsm.tile([128, 1], F32, tag="wsum")
                nc.vector.tensor_reduce(wsum[:, :], mxs[:, :], X, A.add)
                rw = rsm.tile([128, 1], F32, tag="rw")
                nc.vector.reciprocal(rw[:, :], wsum[:, :])
                sel = rsm.tile([128, E], F32, tag="sel")
                nc.vector.scalar_tensor_tensor(
                    out=sel[:, :], in0=wcur[:, :], scalar=-1.0,
                    in1=msk[:, :], op0=A.mult, op1=A.add)
                comb = rsm.tile([128, E], F32, tag="comb")
                nc.vector.tensor_scalar(
                    out=comb[:, :], in0=sel[:, :], scalar1=rw[:, :],
                    scalar2=float(ROUTED_SCALING), op0=A.mult, op1=A.mult)
                for le in range(EPC):
                    nc.tensor.transpose(ct[le][:, ts(t, 128)],
                                        comb[:, le : le + 1], ident[:, :])
            for le in range(EPC):
                row = rsm.tile([1, T], F32, tag="row",
                               name=f"row_{le}")
                nc.vector.tensor_copy(row[:, :], ct[le][:, :])
                nc.gpsimd.partition_broadcast(bc[le][:, :], row[:, :])

        # ---- phase 2: gate/up matmuls + activations ----
        with tc.tile_pool(name="pg_ps", bufs=2, space="PSUM") as pgp, \
             tc.tile_pool(name="pu_ps", bufs=2, space="PSUM") as pup, \
             tc.tile_pool(name="wgb", bufs=8) as wbp, \
             tc.tile_pool(name="silp", bufs=3) as silp:

            def gu_pair(wg_src, wu_src, out_tile, bc_tile):
                # wg_src/wu_src: k -> dram AP [128, 128]; out: [128, T] (low)
                pg = pgp.tile([128, T], F32, tag="pg")
                pu = pup.tile([128, T], F32, tag="pu")
                for k in range(KB):
                    wgb = wbp.tile([128, 128], low, tag="wgb")
                    nc.sync.dma_start(out=wgb[:, :], in_=wg_src(k))
                    for n in range(NH):
                        nc.tensor.matmul(pg[:, ts(n, NS)], lhsT=wgb[:, :],
                                         rhs=xlo[k][:, ts(n, NS)],
                                         start=(k == 0), stop=(k == KB - 1))
                    wub = wbp.tile([128, 128], low, tag="wub")
                    nc.sync.dma_start(out=wub[:, :], in_=wu_src(k))
                    for n in range(NH):
                        nc.tensor.matmul(pu[:, ts(n, NS)], lhsT=wub[:, :],
                                         rhs=xlo[k][:, ts(n, NS)],
                                         start=(k == 0), stop=(k == KB - 1))
                sig = silp.tile([128, T], F32, tag="sig")
                nc.scalar.activation(sig[:, :], pg[:, :], AF.Sigmoid)
                sil = silp.tile([128, T], F32, tag="sil")
                nc.vector.scalar_tensor_tensor(
                    out=sil[:, :], in0=pg[:, :], scalar=0.0,
                    in1=sig[:, :], op0=A.bypass, op1=A.mult)
                if bc_tile is None:
                    nc.vector.scalar_tensor_tensor(
                        out=out_tile[:, :], in0=sil[:, :], scalar=0.0,
                        in1=pu[:, :], op0=A.bypass, op1=A.mult)
                else:
                    tmp = silp.tile([128, T], F32, tag="gutmp")
                    nc.vector.scalar_tensor_tensor(
                        out=tmp[:, :], in0=sil[:, :], scalar=0.0,
                        in1=pu[:, :], op0=A.bypass, op1=A.mult)
                    nc.vector.scalar_tensor_tensor(
                        out=out_tile[:, :], in0=tmp[:, :], scalar=0.0,
                        in1=bc_tile[:, :], op0=A.bypass, op1=A.mult)

            for le in range(EPC):
                for j in range(GJ):
                    gu_pair(
                        lambda k, le=le, j=j: wgu_g_d[le, j, k, :, :],
                        lambda k, le=le, j=j: wgu_u_d[le, j, k, :, :],
                        act[le][j], bc[le])
            for j in range(SJ):
                gu_pair(
                    lambda k, j=j: wsg_g_d[j, k, :, :],
                    lambda k, j=j: wsg_u_d[j, k, :, :],
                    acts[j], None)

        # ---- phase 3: down-projection, accumulate experts + shared ----
        with tc.tile_pool(name="dn_ps", bufs=2, space="PSUM") as dnp, \
             tc.tile_pool(name="wdb", bufs=8) as wdp, \
             tc.tile_pool(name="outp", bufs=3) as outp:
            n_k = EPC * GJ + SJ
            for m in range(MB):
                pd = dnp.tile([128, T], F32, tag="pd")
                i = 0
                for le in range(EPC):
                    for j in range(GJ):
                        wdb = wdp.tile([128, 128], low, tag="wdb")
                        nc.sync.dma_start(out=wdb[:, :],
                                          in_=wd_d[le, m, j, :, :])
                        for n in range(NH):
                            nc.tensor.matmul(
                                pd[:, ts(n, NS)], lhsT=wdb[:, :],
                                rhs=act[le][j][:, ts(n, NS)],
                                start=(i == 0), stop=(i == n_k - 1))
                        i += 1
                for j in range(SJ):
                    wdb = wdp.tile([128, 128], low, tag="wdb")
                    nc.sync.dma_start(out=wdb[:, :], in_=wsd_d[m, j, :, :])
                    for n in range(NH):
                        nc.tensor.matmul(
                            pd[:, ts(n, NS)], lhsT=wdb[:, :],
                            rhs=acts[j][:, ts(n, NS)],
                            start=(i == 0), stop=(i == n_k - 1))
                    i += 1
                osb = outp.tile([128, T], F32, tag="osb")
                nc.vector.tensor_copy(osb[:, :], pd[:, :])
                nc.sync.dma_start(out=part_d[ts(m, 128), :], in_=osb[:, :])

    nc.compile()
    return nc


_CACHE = {}


def _get_nc(low):
    if low not in _CACHE:
        _CACHE[low] = build(low)
    return _CACHE[low]


LOW_DT = mybir.dt.bfloat16
_NP_LO = {F32: np.float32, mybir.dt.bfloat16: ml_dtypes.bfloat16}


def _run(inputs, low=None, trace=False, **kw):
    low = LOW_DT if low is None else low
    nc = _get_nc(low)
    np_lo = _NP_LO[low]
    in_maps = [
        _prep_core(c, inputs["hidden_states"], inputs["w_gate"],
                   inputs["w_gate_up"], inputs["w_down"],
                   inputs["w_shared_gate_up"], inputs["w_shared_down"],
                   np_lo)
        for c in range(N_CORES)
    ]
    res = run_bass_kernel_spmd(nc, in_maps, list(range(N_CORES)),
                               trace=trace, **kw)
    acc = np.zeros((H, T), np.float64)
    for c in range(N_CORES):
        acc += res.results[c]["part"]
    out = np.ascontiguousarray(acc.T).astype(np.float32)
    return out, res


def kernel(**inputs):
    out, _ = _run(inputs)
    return out


# revision 16
# speedup vs baseline: 1.4503x; 1.4503x over previous
"""DeepSeek-style MoE block (grouped top-k routing + 16 routed experts +
shared expert) on 8 Trainium2 NeuronCores.

Sharding: expert-parallel. Core c owns routed experts {2c, 2c+1} (dense
all-token compute, weighted by the combine matrix) plus a 1/8 slice of the
shared expert intermediate dim. Each core computes the full router from a
column-permuted gate matrix so its own experts always land in combine rows
0/1 (keeps the program core-independent). Each core emits an fp32 partial
output [H, T]; partials are summed and transposed on the host.

Math notes:
 - softmax + renormalized top-k weights: the softmax denominator cancels in
   the renormalization, so selection + weights use exp(logit - max) only.
 - ROUTED_SCALING is folded into the combine weights.

All activations/weights are pre-transposed/tiled on the host so every
weight DMA is a single contiguous block and every matmul consumes
[128, 128] stationary slices with [128, T] moving activation slabs.
"""

import sys

sys.path.insert(0, "/opt/trn_rl_repo")

from contextlib import ExitStack

import numpy as np
import ml_dtypes

import concourse.bass as bass
import concourse.mybir as mybir
from concourse import bacc
from concourse.bass import ts
from concourse.tile import TileContext
from concourse.bass_utils import run_bass_kernel_spmd
from concourse.masks import make_identity

F32 = mybir.dt.float32

T, H, E, I = 1024, 2048, 16, 704
IS = 2 * I  # shared expert intermediate
TOP_K, N_GROUP, TOPK_GROUP = 6, 4, 2
ROUTED_SCALING = 2.5

N_CORES = 8
EPC = E // N_CORES  # experts per core (2)
SHI = IS // N_CORES  # shared intermediate slice per core (176)
KB = H // 128  # 16 contraction blocks over hidden dim
GJ = (I + 127) // 128  # 6 col-pair blocks per routed expert
SJ = (SHI + 127) // 128  # 2 col-pair blocks for shared slice
MB = H // 128  # 16 output row blocks
TTB = T // 128  # 8 token tiles


def _expert_perm(c):
    """Permute experts so core c's experts (2c, 2c+1) map to rows 0, 1 while
    preserving the 4-expert group-block structure (group order and
    within-group order are both free)."""
    g = c // 2
    r = (c % 2) * 2
    within = [r, r + 1] + [x for x in range(4) if x not in (r, r + 1)]
    groups = [g] + [x for x in range(N_GROUP) if x != g]
    return [4 * gg + w for gg in groups for w in within]


def _prep_core(c, hs, w_gate, w_gate_up, w_down, w_sgu, w_sd, np_lo):
    f32 = np.float32
    xt = np.ascontiguousarray(np.asarray(hs, f32).T)  # [H, T]
    ins = {"xt32": xt}
    if np_lo != f32:
        ins["xt_lo"] = xt.astype(np_lo)

    perm = _expert_perm(c)
    wg = np.asarray(w_gate, f32)[:, perm]  # [H, E]
    # [128, KB*E]: column k*E+e = w_gate[128k + p, perm[e]]
    ins["wg"] = np.ascontiguousarray(
        wg.reshape(KB, 128, E).transpose(1, 0, 2).reshape(128, KB * E))

    e0 = 2 * c
    wgu = np.asarray(w_gate_up, f32)[e0 : e0 + EPC].astype(np_lo)  # [2,H,2I]
    wdn = np.asarray(w_down, f32)[e0 : e0 + EPC].astype(np_lo)  # [2,I,H]

    # gate/up interleaved blocks: [EPC, GJ, KB, 128, 256] = [g(128) | u(128)]
    wgu_t = np.zeros((EPC, GJ, KB, 128, 256), np_lo)
    # down slabs: [EPC, MB, 128, GJ*128] (row p = concat_j wd[128j+?..] )
    wd_t = np.zeros((EPC, MB, 128, GJ * 128), np_lo)
    for e in range(EPC):
        for j in range(GJ):
            w = min(128, I - 128 * j)
            blk = wgu[e].reshape(KB, 128, 2 * I)
            wgu_t[e, j, :, :, :w] = blk[:, :, 128 * j : 128 * j + w]
            wgu_t[e, j, :, :, 128 : 128 + w] = blk[:, :, I + 128 * j : I + 128 * j + w]
            for m in range(MB):
                wd_t[e, m, :w, 128 * j : 128 * (j + 1)] = \
                    wdn[e, 128 * j : 128 * j + w, 128 * m : 128 * (m + 1)]
    ins["wgu"], ins["wd"] = wgu_t, wd_t

    # shared expert slice: intermediate rows [c*SHI, (c+1)*SHI)
    s0 = c * SHI
    sg = np.asarray(w_sgu, f32)[:, s0 : s0 + SHI].astype(np_lo)
    su = np.asarray(w_sgu, f32)[:, IS + s0 : IS + s0 + SHI].astype(np_lo)
    sd = np.asarray(w_sd, f32)[s0 : s0 + SHI, :].astype(np_lo)

    wsg_t = np.zeros((SJ, KB, 128, 256), np_lo)
    wsd_t = np.zeros((MB, 128, SJ * 128), np_lo)
    for j in range(SJ):
        w = min(128, SHI - 128 * j)
        wsg_t[j, :, :, :w] = sg.reshape(KB, 128, SHI)[:, :, 128 * j : 128 * j + w]
        wsg_t[j, :, :, 128 : 128 + w] = \
            su.reshape(KB, 128, SHI)[:, :, 128 * j : 128 * j + w]
        for m in range(MB):
            wsd_t[m, :w, 128 * j : 128 * (j + 1)] = \
                sd[128 * j : 128 * j + w, 128 * m : 128 * (m + 1)]
    ins["wsg"], ins["wsd"] = wsg_t, wsd_t
    return ins


def build(low=F32, nsplit=None):
    nc = bacc.Bacc("TRN2", target_bir_lowering=False, debug=False,
                   num_devices=N_CORES)
    A = mybir.AluOpType
    X = mybir.AxisListType.X
    AF = mybir.ActivationFunctionType
    # matmul output must stay within one 2KB PSUM bank -> N <= 512 fp32
    if nsplit is None:
        nsplit = 512
    NH = T // nsplit

    xt32_d = nc.dram_tensor("xt32", [H, T], F32, kind="ExternalInput")
    xlo_d = (xt32_d if low == F32 else
             nc.dram_tensor("xt_lo", [H, T], low, kind="ExternalInput"))
    wg_d = nc.dram_tensor("wg", [128, KB * E], F32, kind="ExternalInput")
    wgu_d = nc.dram_tensor("wgu", [EPC, GJ, KB, 128, 256], low,
                           kind="ExternalInput")
    wd_d = nc.dram_tensor("wd", [EPC, MB, 128, GJ * 128], low,
                          kind="ExternalInput")
    wsg_d = nc.dram_tensor("wsg", [SJ, KB, 128, 256], low,
                           kind="ExternalInput")
    wsd_d = nc.dram_tensor("wsd", [MB, 128, SJ * 128], low,
                           kind="ExternalInput")
    part_d = nc.dram_tensor("part", [H, T], F32, kind="ExternalOutput")

    with TileContext(nc) as tc, ExitStack() as ctx:
        ep = ctx.enter_context  # shorthand

        # ---- resident SBUF ----
        xtp = ep(tc.tile_pool(name="xt32p", bufs=KB))
        xt32 = [xtp.tile([128, T], F32, tag="xt32", name=f"xt32_{k}")
                for k in range(KB)]
        for k in range(KB):
            nc.sync.dma_start(out=xt32[k][:, :], in_=xt32_d[ts(k, 128), :])
        if low == F32:
            xlo = xt32
        else:
            xlp = ep(tc.tile_pool(name="xlop", bufs=KB))
            xlo = [xlp.tile([128, T], low, tag="xlo", name=f"xlo_{k}")
                   for k in range(KB)]
            for k in range(KB):
                nc.sync.dma_start(out=xlo[k][:, :], in_=xlo_d[ts(k, 128), :])

        cstp = ep(tc.tile_pool(name="cstp", bufs=1))
        wgsb = cstp.tile([128, KB * E], F32, tag="wgsb")
        nc.sync.dma_start(out=wgsb[:, :], in_=wg_d[:, :])
        ident = cstp.tile([128, 128], F32, tag="ident")
        make_identity(nc, ident[:, :])

        actp = ep(tc.tile_pool(name="actp", bufs=EPC * GJ + SJ))
        act = [[actp.tile([128, T], low, tag="act", name=f"act_{e}_{j}")
                for j in range(GJ)] for e in range(EPC)]
        acts = [actp.tile([128, T], low, tag="act", name=f"acts_{j}")
                for j in range(SJ)]
        bcp = ep(tc.tile_pool(name="bcp", bufs=EPC))
        bc = [bcp.tile([128, T], F32, tag="bc", name=f"bc_{e}")
              for e in range(EPC)]

        # ---- phase 1: router ----
        # logits^T = w_gate^T @ x^T accumulated in one [E, T] psum, then
        # 128-token column blocks are transposed back to token-major tiles.
        with tc.tile_pool(name="lt_ps", bufs=1, space="PSUM") as ltp, \
             tc.tile_pool(name="rt_ps", bufs=2, space="PSUM") as rtp, \
             tc.tile_pool(name="ct_ps", bufs=EPC, space="PSUM") as ctp, \
             tc.tile_pool(name="rsm", bufs=3) as rsm, \
             tc.tile_pool(name="rwk", bufs=3) as rwk:
            lt = ltp.tile([E, T], F32, tag="lt")
            for k in range(KB):
                for n in range(2):
                    nc.tensor.matmul(lt[:, ts(n, 512)],
                                     lhsT=wgsb[:, ts(k, E)],
                                     rhs=xt32[k][:, ts(n, 512)],
                                     start=(k == 0), stop=(k == KB - 1))
            lts = rsm.tile([E, T], F32, tag="lts")
            nc.vector.tensor_copy(lts[:, :], lt[:, :])
            ct = [ctp.tile([1, T], F32, tag="ct", name=f"ct_{e}")
                  for e in range(EPC)]
            for t in range(TTB):
                pl = rtp.tile([128, E], F32, tag="pl")
                nc.tensor.transpose(pl[:, :], lts[:, ts(t, 128)],
                                    ident[0:E, 0:E])
                nm = rsm.tile([128, 1], F32, tag="nm")
                nc.vector.tensor_reduce(nm[:, :], pl[:, :], X, A.max,
                                        negate=True)
                es = rsm.tile([128, E], F32, tag="es")
                nc.scalar.activation(es[:, :], pl[:, :], AF.Exp, bias=nm[:, :])
                gmax = rsm.tile([128, N_GROUP], F32, tag="gmax")
                nc.vector.tensor_reduce(
                    gmax[:, :],
                    es[:, :].rearrange("p (g e) -> p g e", g=N_GROUP),
                    X, A.max)
                m1 = rsm.tile([128, 1], F32, tag="m1")
                nc.vector.tensor_reduce(m1[:, :], gmax[:, :], X, A.max)
                gz = rsm.tile([128, N_GROUP], F32, tag="gz")
                nc.vector.scalar_tensor_tensor(
                    out=gz[:, :], in0=gmax[:, :], scalar=m1[:, :],
                    in1=gmax[:, :], op0=A.is_lt, op1=A.mult)
                m2 = rsm.tile([128, 1], F32, tag="m2")
                nc.vector.tensor_reduce(m2[:, :], gz[:, :], X, A.max)
                keep = rsm.tile([128, N_GROUP], F32, tag="keep")
                nc.vector.tensor_scalar(
                    out=keep[:, :], in0=gmax[:, :], scalar1=m2[:, :],
                    scalar2=None, op0=A.is_ge)
                msk = rsm.tile([128, E], F32, tag="msk")
                for g in range(N_GROUP):
                    nc.vector.tensor_scalar(
                        out=msk[:, 4 * g : 4 * g + 4],
                        in0=es[:, 4 * g : 4 * g + 4],
                        scalar1=keep[:, g : g + 1], scalar2=None, op0=A.mult)
                mxs = rsm.tile([128, TOP_K], F32, tag="mxs")
                wcur = msk
                for i in range(TOP_K):
                    nc.vector.tensor_reduce(mxs[:, i : i + 1], wcur[:, :],
                                            X, A.max)
                    wnxt = rwk.tile([128, E], F32, tag="wk")
                    nc.vector.scalar_tensor_tensor(
                        out=wnxt[:, :], in0=wcur[:, :],
                        scalar=mxs[:, i : i + 1], in1=wcur[:, :],
                        op0=A.is_lt, op1=A.mult)
                    wcur = wnxt
                wsum = rsm.tile([128, 1], F32, tag="wsum")
                nc.vector.tensor_reduce(wsum[:, :], mxs[:, :], X, A.add)
                rw = rsm.tile([128, 1], F32, tag="rw")
                nc.vector.reciprocal(rw[:, :], wsum[:, :])
                sel = rsm.tile([128, E], F32, tag="sel")
                nc.vector.scalar_tensor_tensor(
                    out=sel[:, :], in0=wcur[:, :], scalar=-1.0,
                    in1=msk[:, :], op0=A.mult, op1=A.add)
                comb = rsm.tile([128, E], F32, tag="comb")
                nc.vector.tensor_scalar(
                    out=comb[:, :], in0=sel[:, :], scalar1=rw[:, :],
                    scalar2=float(ROUTED_SCALING), op0=A.mult, op1=A.mult)
                for le in range(EPC):
                    nc.tensor.transpose(ct[le][:, ts(t, 128)],
                                        comb[:, le : le + 1], ident[:, :])
            for le in range(EPC):
                row = rsm.tile([1, T], F32, tag="row",
                               name=f"row_{le}")
                nc.vector.tensor_copy(row[:, :], ct[le][:, :])
                nc.gpsimd.partition_broadcast(bc[le][:, :], row[:, :])

        # ---- phase 2: gate/up matmuls + activations ----
        with tc.tile_pool(name="pg_ps", bufs=2, space="PSUM") as pgp, \
             tc.tile_pool(name="pu_ps", bufs=2, space="PSUM") as pup, \
             tc.tile_pool(name="wgb", bufs=6) as wbp, \
             tc.tile_pool(name="silp", bufs=2) as silp:

            def gu_pair(w_src, out_tile, bc_tile):
                # w_src: k -> dram AP [128, 256] ([g|u] block)
                pg = pgp.tile([128, T], F32, tag="pg")
                pu = pup.tile([128, T], F32, tag="pu")
                for k in range(KB):
                    wb = wbp.tile([128, 256], low, tag="wb")
                    nc.sync.dma_start(out=wb[:, :], in_=w_src(k))
                    for n in range(NH):
                        nc.tensor.matmul(pg[:, ts(n, nsplit)],
                                         lhsT=wb[:, 0:128],
                                         rhs=xlo[k][:, ts(n, nsplit)],
                                         start=(k == 0), stop=(k == KB - 1))
                    for n in range(NH):
                        nc.tensor.matmul(pu[:, ts(n, nsplit)],
                                         lhsT=wb[:, 128:256],
                                         rhs=xlo[k][:, ts(n, nsplit)],
                                         start=(k == 0), stop=(k == KB - 1))
                sig = silp.tile([128, T], F32, tag="sig")
                nc.scalar.activation(sig[:, :], pg[:, :], AF.Sigmoid)
                sil = silp.tile([128, T], F32, tag="sil")
                nc.vector.scalar_tensor_tensor(
                    out=sil[:, :], in0=pg[:, :], scalar=0.0,
                    in1=sig[:, :], op0=A.bypass, op1=A.mult)
                if bc_tile is None:
                    nc.vector.scalar_tensor_tensor(
                        out=out_tile[:, :], in0=sil[:, :], scalar=0.0,
                        in1=pu[:, :], op0=A.bypass, op1=A.mult)
                else:
                    tmp = silp.tile([128, T], F32, tag="gutmp")
                    nc.vector.scalar_tensor_tensor(
                        out=tmp[:, :], in0=sil[:, :], scalar=0.0,
                        in1=pu[:, :], op0=A.bypass, op1=A.mult)
                    nc.vector.scalar_tensor_tensor(
                        out=out_tile[:, :], in0=tmp[:, :], scalar=0.0,
                        in1=bc_tile[:, :], op0=A.bypass, op1=A.mult)

            for le in range(EPC):
                for j in range(GJ):
                    gu_pair(lambda k, le=le, j=j: wgu_d[le, j, k, :, :],
                            act[le][j], bc[le])
            for j in range(SJ):
                gu_pair(lambda k, j=j: wsg_d[j, k, :, :], acts[j], None)

        # ---- phase 3: down-projection, accumulate experts + shared ----
        with tc.tile_pool(name="dn_ps", bufs=2, space="PSUM") as dnp, \
             tc.tile_pool(name="wdp", bufs=4) as wdp, \
             tc.tile_pool(name="wsp", bufs=2) as wsp, \
             tc.tile_pool(name="outp", bufs=3) as outp:
            n_k = EPC * GJ + SJ
            for m in range(MB):
                pd = dnp.tile([128, T], F32, tag="pd")
                slabs = [wdp.tile([128, GJ * 128], low, tag="wdslab",
                                  name=f"wds_{m}_{le}")
                         for le in range(EPC)]
                for le in range(EPC):
                    nc.sync.dma_start(out=slabs[le][:, :],
                                      in_=wd_d[le, m, :, :])
                sslab = wsp.tile([128, SJ * 128], low, tag="wsslab")
                nc.sync.dma_start(out=sslab[:, :], in_=wsd_d[m, :, :])
                i = 0
                for le in range(EPC):
                    for j in range(GJ):
                        for n in range(NH):
                            nc.tensor.matmul(
                                pd[:, ts(n, nsplit)],
                                lhsT=slabs[le][:, ts(j, 128)],
                                rhs=act[le][j][:, ts(n, nsplit)],
                                start=(i == 0), stop=(i == n_k - 1))
                        i += 1
                for j in range(SJ):
                    for n in range(NH):
                        nc.tensor.matmul(
                            pd[:, ts(n, nsplit)],
                            lhsT=sslab[:, ts(j, 128)],
                            rhs=acts[j][:, ts(n, nsplit)],
                            start=(i == 0), stop=(i == n_k - 1))
                    i += 1
                osb = outp.tile([128, T], F32, tag="osb")
                nc.vector.tensor_copy(osb[:, :], pd[:, :])
                nc.sync.dma_start(out=part_d[ts(m, 128), :], in_=osb[:, :])

    nc.compile()
    return nc


_CACHE = {}


def _get_nc(low):
    if low not in _CACHE:
        _CACHE[low] = build(low)
    return _CACHE[low]


LOW_DT = mybir.dt.bfloat16
_NP_LO = {F32: np.float32, mybir.dt.bfloat16: ml_dtypes.bfloat16}


def _run(inputs, low=None, trace=False, **kw):
    low = LOW_DT if low is None else low
    nc = _get_nc(low)
    np_lo = _NP_LO[low]
    in_maps = [
        _prep_core(c, inputs["hidden_states"], inputs["w_gate"],
                   inputs["w_gate_up"], inputs["w_down"],
                   inputs["w_shared_gate_up"], inputs["w_shared_down"],
                   np_lo)
        for c in range(N_CORES)
    ]
    res = run_bass_kernel_spmd(nc, in_maps, list(range(N_CORES)),
                               trace=trace, **kw)
    acc = np.zeros((H, T), np.float64)
    for c in range(N_CORES):
        acc += res.results[c]["part"]
    out = np.ascontiguousarray(acc.T).astype(np.float32)
    return out, res


def kernel(**inputs):
    out, _ = _run(inputs)
    return out


# revision 18
# speedup vs baseline: 1.5268x; 1.0527x over previous
"""DeepSeek-style MoE block (grouped top-k routing + 16 routed experts +
shared expert) on 8 Trainium2 NeuronCores.

Sharding: expert-parallel. Core c owns routed experts {2c, 2c+1} (dense
all-token compute, weighted by the combine matrix) plus a 1/8 slice of the
shared expert intermediate dim. Each core computes the full router from a
column-permuted gate matrix so its own experts always land in combine rows
0/1 (keeps the program core-independent). Each core emits an fp32 partial
output [H, T]; partials are summed and transposed on the host.

Math notes:
 - softmax + renormalized top-k weights: the softmax denominator cancels in
   the renormalization, so selection + weights use exp(logit - max) only.
 - ROUTED_SCALING is folded into the combine weights.

All activations/weights are pre-transposed/tiled on the host so every
weight DMA is a single contiguous block and every matmul consumes
[128, 128] stationary slices with [128, T] moving activation slabs.
"""

import sys

sys.path.insert(0, "/opt/trn_rl_repo")

from contextlib import ExitStack

import numpy as np
import ml_dtypes

import concourse.bass as bass
import concourse.mybir as mybir
from concourse import bacc
from concourse.bass import ts
from concourse.tile import TileContext
from concourse.bass_utils import run_bass_kernel_spmd
from concourse.masks import make_identity

F32 = mybir.dt.float32

T, H, E, I = 1024, 2048, 16, 704
IS = 2 * I  # shared expert intermediate
TOP_K, N_GROUP, TOPK_GROUP = 6, 4, 2
ROUTED_SCALING = 2.5

N_CORES = 8
EPC = E // N_CORES  # experts per core (2)
SHI = IS // N_CORES  # shared intermediate slice per core (176)
KB = H // 128  # 16 contraction blocks over hidden dim
GJ = (I + 127) // 128  # 6 col-pair blocks per routed expert
SJ = (SHI + 127) // 128  # 2 col-pair blocks for shared slice
MB = H // 128  # 16 output row blocks
TTB = T // 128  # 8 token tiles


def _expert_perm(c):
    """Permute experts so core c's experts (2c, 2c+1) map to rows 0, 1 while
    preserving the 4-expert group-block structure (group order and
    within-group order are both free)."""
    g = c // 2
    r = (c % 2) * 2
    within = [r, r + 1] + [x for x in range(4) if x not in (r, r + 1)]
    groups = [g] + [x for x in range(N_GROUP) if x != g]
    return [4 * gg + w for gg in groups for w in within]


def _prep_core(c, hs, w_gate, w_gate_up, w_down, w_sgu, w_sd, np_lo):
    f32 = np.float32
    xt = np.ascontiguousarray(np.asarray(hs, f32).T)  # [H, T]
    ins = {"xt32": xt}
    if np_lo != f32:
        ins["xt_lo"] = xt.astype(np_lo)

    perm = _expert_perm(c)
    wg = np.asarray(w_gate, f32)[:, perm]  # [H, E]
    # [128, KB*E]: column k*E+e = w_gate[128k + p, perm[e]]
    ins["wg"] = np.ascontiguousarray(
        wg.reshape(KB, 128, E).transpose(1, 0, 2).reshape(128, KB * E))

    e0 = 2 * c
    wgu = np.asarray(w_gate_up, f32)[e0 : e0 + EPC].astype(np_lo)  # [2,H,2I]
    wdn = np.asarray(w_down, f32)[e0 : e0 + EPC].astype(np_lo)  # [2,I,H]

    # gate/up interleaved blocks: [EPC, GJ, KB, 128, 256] = [g(128) | u(128)]
    wgu_t = np.zeros((EPC, GJ, KB, 128, 256), np_lo)
    # down slabs: [EPC, MB, 128, GJ*128] (row p = concat_j wd[128j+?..] )
    wd_t = np.zeros((EPC, MB, 128, GJ * 128), np_lo)
    for e in range(EPC):
        for j in range(GJ):
            w = min(128, I - 128 * j)
            blk = wgu[e].reshape(KB, 128, 2 * I)
            wgu_t[e, j, :, :, :w] = blk[:, :, 128 * j : 128 * j + w]
            wgu_t[e, j, :, :, 128 : 128 + w] = blk[:, :, I + 128 * j : I + 128 * j + w]
            for m in range(MB):
                wd_t[e, m, :w, 128 * j : 128 * (j + 1)] = \
                    wdn[e, 128 * j : 128 * j + w, 128 * m : 128 * (m + 1)]
    ins["wgu"], ins["wd"] = wgu_t, wd_t

    # shared expert slice: intermediate rows [c*SHI, (c+1)*SHI)
    s0 = c * SHI
    sg = np.asarray(w_sgu, f32)[:, s0 : s0 + SHI].astype(np_lo)
    su = np.asarray(w_sgu, f32)[:, IS + s0 : IS + s0 + SHI].astype(np_lo)
    sd = np.asarray(w_sd, f32)[s0 : s0 + SHI, :].astype(np_lo)

    wsg_t = np.zeros((SJ, KB, 128, 256), np_lo)
    wsd_t = np.zeros((MB, 128, SJ * 128), np_lo)
    for j in range(SJ):
        w = min(128, SHI - 128 * j)
        wsg_t[j, :, :, :w] = sg.reshape(KB, 128, SHI)[:, :, 128 * j : 128 * j + w]
        wsg_t[j, :, :, 128 : 128 + w] = \
            su.reshape(KB, 128, SHI)[:, :, 128 * j : 128 * j + w]
        for m in range(MB):
            wsd_t[m, :w, 128 * j : 128 * (j + 1)] = \
                sd[128 * j : 128 * j + w, 128 * m : 128 * (m + 1)]
    ins["wsg"], ins["wsd"] = wsg_t, wsd_t
    return ins


def build(low=F32, nsplit=None):
    nc = bacc.Bacc("TRN2", target_bir_lowering=False, debug=False,
                   num_devices=N_CORES)
    A = mybir.AluOpType
    X = mybir.AxisListType.X
    AF = mybir.ActivationFunctionType
    # matmul output must stay within one 2KB PSUM bank -> N <= 512 fp32
    if nsplit is None:
        nsplit = 512
    NH = T // nsplit

    xt32_d = nc.dram_tensor("xt32", [H, T], F32, kind="ExternalInput")
    xlo_d = (xt32_d if low == F32 else
             nc.dram_tensor("xt_lo", [H, T], low, kind="ExternalInput"))
    wg_d = nc.dram_tensor("wg", [128, KB * E], F32, kind="ExternalInput")
    wgu_d = nc.dram_tensor("wgu", [EPC, GJ, KB, 128, 256], low,
                           kind="ExternalInput")
    wd_d = nc.dram_tensor("wd", [EPC, MB, 128, GJ * 128], low,
                          kind="ExternalInput")
    wsg_d = nc.dram_tensor("wsg", [SJ, KB, 128, 256], low,
                           kind="ExternalInput")
    wsd_d = nc.dram_tensor("wsd", [MB, 128, SJ * 128], low,
                           kind="ExternalInput")
    part_d = nc.dram_tensor("part", [H, T], F32, kind="ExternalOutput")

    with TileContext(nc) as tc, ExitStack() as ctx:
        ep = ctx.enter_context  # shorthand

        # ---- resident SBUF ----
        xtp = ep(tc.tile_pool(name="xt32p", bufs=KB))
        xt32 = [xtp.tile([128, T], F32, tag="xt32", name=f"xt32_{k}")
                for k in range(KB)]
        for k in range(KB):
            nc.sync.dma_start(out=xt32[k][:, :], in_=xt32_d[ts(k, 128), :])
        if low == F32:
            xlo = xt32
        else:
            xlp = ep(tc.tile_pool(name="xlop", bufs=KB))
            xlo = [xlp.tile([128, T], low, tag="xlo", name=f"xlo_{k}")
                   for k in range(KB)]
            for k in range(KB):
                nc.sync.dma_start(out=xlo[k][:, :], in_=xlo_d[ts(k, 128), :])

        cstp = ep(tc.tile_pool(name="cstp", bufs=1))
        wgsb = cstp.tile([128, KB * E], F32, tag="wgsb")
        nc.sync.dma_start(out=wgsb[:, :], in_=wg_d[:, :])
        ident = cstp.tile([128, 128], F32, tag="ident")
        make_identity(nc, ident[:, :])

        actp = ep(tc.tile_pool(name="actp", bufs=EPC * GJ + SJ))
        act = [[actp.tile([128, T], low, tag="act", name=f"act_{e}_{j}")
                for j in range(GJ)] for e in range(EPC)]
        acts = [actp.tile([128, T], low, tag="act", name=f"acts_{j}")
                for j in range(SJ)]
        bcp = ep(tc.tile_pool(name="bcp", bufs=EPC))
        bc = [bcp.tile([128, T], F32, tag="bc", name=f"bc_{e}")
              for e in range(EPC)]

        # gate/up PSUM pools first so they claim banks 0-3; the router's
        # pools live in banks 4-7 and never block the expert matmuls.
        with tc.tile_pool(name="pg_ps", bufs=2, space="PSUM") as pgp, \
             tc.tile_pool(name="pu_ps", bufs=2, space="PSUM") as pup, \
             tc.tile_pool(name="wgb", bufs=6) as wbp, \
             tc.tile_pool(name="silp", bufs=3) as silp:

            # ---- phase 1: router ----
            # logits^T = w_gate^T @ x^T accumulated in one [E, T] psum,
            # transposed back to token-major 128-token tiles on the PE.
            with tc.tile_pool(name="rt_ps", bufs=2, space="PSUM") as rtp, \
                 tc.tile_pool(name="rsm", bufs=3) as rsm, \
                 tc.tile_pool(name="rwk", bufs=3) as rwk:
                with tc.tile_pool(name="lt_ps", bufs=1, space="PSUM") as ltp:
                    lt = ltp.tile([E, T], F32, tag="lt")
                    for k in range(KB):
                        for n in range(2):
                            nc.tensor.matmul(lt[:, ts(n, 512)],
                                             lhsT=wgsb[:, ts(k, E)],
                                             rhs=xt32[k][:, ts(n, 512)],
                                             start=(k == 0),
                                             stop=(k == KB - 1))
                    lts = rsm.tile([E, T], F32, tag="lts")
                    nc.vector.tensor_copy(lts[:, :], lt[:, :])
                with tc.tile_pool(name="ct_ps", bufs=1, space="PSUM") as ctp:
                    ct2 = ctp.tile([EPC, T], F32, tag="ct2")
                    for t in range(TTB):
                        pl = rtp.tile([128, E], F32, tag="pl")
                        nc.tensor.transpose(pl[:, :], lts[:, ts(t, 128)],
                                            ident[0:E, 0:E])
                        nm = rsm.tile([128, 1], F32, tag="nm")
                        nc.vector.tensor_reduce(nm[:, :], pl[:, :], X, A.max,
                                                negate=True)
                        es = rsm.tile([128, E], F32, tag="es")
                        nc.scalar.activation(es[:, :], pl[:, :], AF.Exp,
                                             bias=nm[:, :])
                        gmax = rsm.tile([128, N_GROUP], F32, tag="gmax")
                        nc.vector.tensor_reduce(
                            gmax[:, :],
                            es[:, :].rearrange("p (g e) -> p g e", g=N_GROUP),
                            X, A.max)
                        m1 = rsm.tile([128, 1], F32, tag="m1")
                        nc.vector.tensor_reduce(m1[:, :], gmax[:, :], X, A.max)
                        gz = rsm.tile([128, N_GROUP], F32, tag="gz")
                        nc.vector.scalar_tensor_tensor(
                            out=gz[:, :], in0=gmax[:, :], scalar=m1[:, :],
                            in1=gmax[:, :], op0=A.is_lt, op1=A.mult)
                        m2 = rsm.tile([128, 1], F32, tag="m2")
                        nc.vector.tensor_reduce(m2[:, :], gz[:, :], X, A.max)
                        keep = rsm.tile([128, N_GROUP], F32, tag="keep")
                        nc.vector.tensor_scalar(
                            out=keep[:, :], in0=gmax[:, :], scalar1=m2[:, :],
                            scalar2=None, op0=A.is_ge)
                        msk = rsm.tile([128, E], F32, tag="msk")
                        for g in range(N_GROUP):
                            nc.vector.tensor_scalar(
                                out=msk[:, 4 * g : 4 * g + 4],
                                in0=es[:, 4 * g : 4 * g + 4],
                                scalar1=keep[:, g : g + 1], scalar2=None,
                                op0=A.mult)
                        mxs = rsm.tile([128, TOP_K], F32, tag="mxs")
                        wcur = msk
                        for i in range(TOP_K):
                            nc.vector.tensor_reduce(mxs[:, i : i + 1],
                                                    wcur[:, :], X, A.max)
                            wnxt = rwk.tile([128, E], F32, tag="wk")
                            nc.vector.scalar_tensor_tensor(
                                out=wnxt[:, :], in0=wcur[:, :],
                                scalar=mxs[:, i : i + 1], in1=wcur[:, :],
                                op0=A.is_lt, op1=A.mult)
                            wcur = wnxt
                        wsum = rsm.tile([128, 1], F32, tag="wsum")
                        nc.vector.tensor_reduce(wsum[:, :], mxs[:, :], X,
                                                A.add)
                        rw = rsm.tile([128, 1], F32, tag="rw")
                        nc.vector.reciprocal(rw[:, :], wsum[:, :])
                        sel = rsm.tile([128, E], F32, tag="sel")
                        nc.vector.scalar_tensor_tensor(
                            out=sel[:, :], in0=wcur[:, :], scalar=-1.0,
                            in1=msk[:, :], op0=A.mult, op1=A.add)
                        comb = rsm.tile([128, E], F32, tag="comb")
                        nc.vector.tensor_scalar(
                            out=comb[:, :], in0=sel[:, :], scalar1=rw[:, :],
                            scalar2=float(ROUTED_SCALING), op0=A.mult,
                            op1=A.mult)
                        nc.tensor.transpose(ct2[:, ts(t, 128)],
                                            comb[:, 0:EPC], ident[:, :])
                    rows = rsm.tile([32, T], F32, tag="rows")
                    nc.vector.memset(rows[:, :], 0.0)
                    nc.vector.tensor_copy(rows[0:EPC, :], ct2[:, :])
                rowsb = rsm.tile([32, T], F32, tag="rowsb")
                nc.vector.stream_shuffle(rowsb[:, :], rows[:, :],
                                         mask=[1] + list(range(1, 32)))
                nc.gpsimd.partition_broadcast(bc[0][:, :], rows[0:1, :])
                nc.gpsimd.partition_broadcast(bc[1][:, :], rowsb[0:1, :])

            # ---- phase 2: gate/up matmuls + activations ----
            def gu_pair(w_src, out_tile, bc_tile):
                # w_src: k -> dram AP [128, 256] ([g|u] block)
                pgh = [pgp.tile([128, nsplit], F32, tag="pg",
                                name=f"pg_{h}") for h in range(NH)]
                puh = [pup.tile([128, nsplit], F32, tag="pu",
                                name=f"pu_{h}") for h in range(NH)]
                for k in range(KB):
                    wb = wbp.tile([128, 256], low, tag="wb")
                    nc.sync.dma_start(out=wb[:, :], in_=w_src(k))
                    for h in range(NH):
                        nc.tensor.matmul(pgh[h][:, :], lhsT=wb[:, 0:128],
                                         rhs=xlo[k][:, ts(h, nsplit)],
                                         start=(k == 0), stop=(k == KB - 1))
                    for h in range(NH):
                        nc.tensor.matmul(puh[h][:, :], lhsT=wb[:, 128:256],
                                         rhs=xlo[k][:, ts(h, nsplit)],
                                         start=(k == 0), stop=(k == KB - 1))
                for h in range(NH):
                    hs_ = ts(h, nsplit)
                    sig = silp.tile([128, nsplit], F32, tag="sig")
                    nc.scalar.activation(sig[:, :], pgh[h][:, :], AF.Sigmoid)
                    sil = silp.tile([128, nsplit], F32, tag="sil")
                    nc.vector.scalar_tensor_tensor(
                        out=sil[:, :], in0=pgh[h][:, :], scalar=0.0,
                        in1=sig[:, :], op0=A.bypass, op1=A.mult)
                    if bc_tile is None:
                        nc.vector.scalar_tensor_tensor(
                            out=out_tile[:, hs_], in0=sil[:, :], scalar=0.0,
                            in1=puh[h][:, :], op0=A.bypass, op1=A.mult)
                    else:
                        tmp = silp.tile([128, nsplit], F32, tag="gutmp")
                        nc.vector.scalar_tensor_tensor(
                            out=tmp[:, :], in0=sil[:, :], scalar=0.0,
                            in1=puh[h][:, :], op0=A.bypass, op1=A.mult)
                        nc.vector.scalar_tensor_tensor(
                            out=out_tile[:, hs_], in0=tmp[:, :], scalar=0.0,
                            in1=bc_tile[:, hs_], op0=A.bypass, op1=A.mult)

            for le in range(EPC):
                for j in range(GJ):
                    gu_pair(lambda k, le=le, j=j: wgu_d[le, j, k, :, :],
                            act[le][j], bc[le])
            for j in range(SJ):
                gu_pair(lambda k, j=j: wsg_d[j, k, :, :], acts[j], None)

            # ---- phase 3: down-projection (psum banks 4-7) ----
            with tc.tile_pool(name="dn_ps", bufs=2, space="PSUM") as dnp, \
                 tc.tile_pool(name="wdp", bufs=4) as wdp, \
                 tc.tile_pool(name="wsp", bufs=2) as wsp, \
                 tc.tile_pool(name="outp", bufs=3) as outp:
                n_k = EPC * GJ + SJ
                for m in range(MB):
                    pd = dnp.tile([128, T], F32, tag="pd")
                    slabs = [wdp.tile([128, GJ * 128], low, tag="wdslab",
                                      name=f"wds_{m}_{le}")
                             for le in range(EPC)]
                    for le in range(EPC):
                        nc.sync.dma_start(out=slabs[le][:, :],
                                          in_=wd_d[le, m, :, :])
                    sslab = wsp.tile([128, SJ * 128], low, tag="wsslab")
                    nc.sync.dma_start(out=sslab[:, :], in_=wsd_d[m, :, :])
                    i = 0
                    for le in range(EPC):
                        for j in range(GJ):
                            for n in range(NH):
                                nc.tensor.matmul(
                                    pd[:, ts(n, nsplit)],
                                    lhsT=slabs[le][:, ts(j, 128)],
                                    rhs=act[le][j][:, ts(n, nsplit)],
                                    start=(i == 0), stop=(i == n_k - 1))
                            i += 1
                    for j in range(SJ):
                        for n in range(NH):
                            nc.tensor.matmul(
                                pd[:, ts(n, nsplit)],
                                lhsT=sslab[:, ts(j, 128)],
                                rhs=acts[j][:, ts(n, nsplit)],
                                start=(i == 0), stop=(i == n_k - 1))
                        i += 1
                    osb = outp.tile([128, T], F32, tag="osb")
                    nc.vector.tensor_copy(osb[:, :], pd[:, :])
                    nc.sync.dma_start(out=part_d[ts(m, 128), :],
                                      in_=osb[:, :])

    nc.compile()
    return nc


_CACHE = {}


def _get_nc(low):
    if low not in _CACHE:
        _CACHE[low] = build(low)
    return _CACHE[low]


LOW_DT = mybir.dt.bfloat16
_NP_LO = {F32: np.float32, mybir.dt.bfloat16: ml_dtypes.bfloat16}


def _run(inputs, low=None, trace=False, **kw):
    low = LOW_DT if low is None else low
    nc = _get_nc(low)
    np_lo = _NP_LO[low]
    in_maps = [
        _prep_core(c, inputs["hidden_states"], inputs["w_gate"],
                   inputs["w_gate_up"], inputs["w_down"],
                   inputs["w_shared_gate_up"], inputs["w_shared_down"],
                   np_lo)
        for c in range(N_CORES)
    ]
    res = run_bass_kernel_spmd(nc, in_maps, list(range(N_CORES)),
                               trace=trace, **kw)
    acc = np.zeros((H, T), np.float64)
    for c in range(N_CORES):
        acc += res.results[c]["part"]
    out = np.ascontiguousarray(acc.T).astype(np.float32)
    return out, res


def kernel(**inputs):
    out, _ = _run(inputs)
    return out


# revision 19
# speedup vs baseline: 1.6167x; 1.0589x over previous
"""DeepSeek-style MoE block (grouped top-k routing + 16 routed experts +
shared expert) on 8 Trainium2 NeuronCores.

Sharding: expert-parallel. Core c owns routed experts {2c, 2c+1} (dense
all-token compute, weighted by the combine matrix) plus a 1/8 slice of the
shared expert intermediate dim. Each core computes the full router from a
column-permuted gate matrix so its own experts always land in combine rows
0/1 (keeps the program core-independent). Each core emits an fp32 partial
output [H, T]; partials are summed and transposed on the host.

Math notes:
 - softmax + renormalized top-k weights: the softmax denominator cancels in
   the renormalization, so selection + weights use exp(logit - max) only.
 - ROUTED_SCALING is folded into the combine weights.

All activations/weights are pre-transposed/tiled on the host so every
weight DMA is a single contiguous block and every matmul consumes
[128, 128] stationary slices with [128, T] moving activation slabs.
"""

import sys

sys.path.insert(0, "/opt/trn_rl_repo")

from contextlib import ExitStack

import numpy as np
import ml_dtypes

import concourse.bass as bass
import concourse.mybir as mybir
from concourse import bacc
from concourse.bass import ts
from concourse.tile import TileContext
from concourse.bass_utils import run_bass_kernel_spmd
from concourse.masks import make_identity

F32 = mybir.dt.float32

T, H, E, I = 1024, 2048, 16, 704
IS = 2 * I  # shared expert intermediate
TOP_K, N_GROUP, TOPK_GROUP = 6, 4, 2
ROUTED_SCALING = 2.5

N_CORES = 8
EPC = E // N_CORES  # experts per core (2)
SHI = IS // N_CORES  # shared intermediate slice per core (176)
KB = H // 128  # 16 contraction blocks over hidden dim
GJ = (I + 127) // 128  # 6 col-pair blocks per routed expert
SJ = (SHI + 127) // 128  # 2 col-pair blocks for shared slice
MB = H // 128  # 16 output row blocks
TTB = T // 128  # 8 token tiles


def _expert_perm(c):
    """Permute experts so core c's experts (2c, 2c+1) map to rows 0, 1 while
    preserving the 4-expert group-block structure (group order and
    within-group order are both free)."""
    g = c // 2
    r = (c % 2) * 2
    within = [r, r + 1] + [x for x in range(4) if x not in (r, r + 1)]
    groups = [g] + [x for x in range(N_GROUP) if x != g]
    return [4 * gg + w for gg in groups for w in within]


def _prep_core(c, hs, w_gate, w_gate_up, w_down, w_sgu, w_sd, np_lo):
    f32 = np.float32
    xt = np.ascontiguousarray(np.asarray(hs, f32).T)  # [H, T]
    ins = {"xt32": xt}
    if np_lo != f32:
        ins["xt_lo"] = xt.astype(np_lo)

    perm = _expert_perm(c)
    wg = np.asarray(w_gate, f32)[:, perm]  # [H, E]
    # [128, KB*E]: column k*E+e = w_gate[128k + p, perm[e]]
    ins["wg"] = np.ascontiguousarray(
        wg.reshape(KB, 128, E).transpose(1, 0, 2).reshape(128, KB * E))

    e0 = 2 * c
    wgu = np.asarray(w_gate_up, f32)[e0 : e0 + EPC].astype(np_lo)  # [2,H,2I]
    wdn = np.asarray(w_down, f32)[e0 : e0 + EPC].astype(np_lo)  # [2,I,H]

    # gate/up interleaved blocks: [EPC, GJ, KB, 128, 256] = [g(128) | u(128)]
    wgu_t = np.zeros((EPC, GJ, KB, 128, 256), np_lo)
    # down slabs: [EPC, MB, 128, GJ*128] (row p = concat_j wd[128j+?..] )
    wd_t = np.zeros((EPC, MB, 128, GJ * 128), np_lo)
    for e in range(EPC):
        for j in range(GJ):
            w = min(128, I - 128 * j)
            blk = wgu[e].reshape(KB, 128, 2 * I)
            wgu_t[e, j, :, :, :w] = blk[:, :, 128 * j : 128 * j + w]
            wgu_t[e, j, :, :, 128 : 128 + w] = blk[:, :, I + 128 * j : I + 128 * j + w]
            for m in range(MB):
                wd_t[e, m, :w, 128 * j : 128 * (j + 1)] = \
                    wdn[e, 128 * j : 128 * j + w, 128 * m : 128 * (m + 1)]
    ins["wgu"], ins["wd"] = wgu_t, wd_t

    # shared expert slice: intermediate rows [c*SHI, (c+1)*SHI)
    s0 = c * SHI
    sg = np.asarray(w_sgu, f32)[:, s0 : s0 + SHI].astype(np_lo)
    su = np.asarray(w_sgu, f32)[:, IS + s0 : IS + s0 + SHI].astype(np_lo)
    sd = np.asarray(w_sd, f32)[s0 : s0 + SHI, :].astype(np_lo)

    wsg_t = np.zeros((SJ, KB, 128, 256), np_lo)
    wsd_t = np.zeros((MB, 128, SJ * 128), np_lo)
    for j in range(SJ):
        w = min(128, SHI - 128 * j)
        wsg_t[j, :, :, :w] = sg.reshape(KB, 128, SHI)[:, :, 128 * j : 128 * j + w]
        wsg_t[j, :, :, 128 : 128 + w] = \
            su.reshape(KB, 128, SHI)[:, :, 128 * j : 128 * j + w]
        for m in range(MB):
            wsd_t[m, :w, 128 * j : 128 * (j + 1)] = \
                sd[128 * j : 128 * j + w, 128 * m : 128 * (m + 1)]
    ins["wsg"], ins["wsd"] = wsg_t, wsd_t
    return ins


def build(low=F32, nsplit=None):
    nc = bacc.Bacc("TRN2", target_bir_lowering=False, debug=False,
                   num_devices=N_CORES)
    A = mybir.AluOpType
    X = mybir.AxisListType.X
    AF = mybir.ActivationFunctionType
    # matmul output must stay within one 2KB PSUM bank -> N <= 512 fp32
    if nsplit is None:
        nsplit = 512
    NH = T // nsplit

    xt32_d = nc.dram_tensor("xt32", [H, T], F32, kind="ExternalInput")
    xlo_d = (xt32_d if low == F32 else
             nc.dram_tensor("xt_lo", [H, T], low, kind="ExternalInput"))
    wg_d = nc.dram_tensor("wg", [128, KB * E], F32, kind="ExternalInput")
    wgu_d = nc.dram_tensor("wgu", [EPC, GJ, KB, 128, 256], low,
                           kind="ExternalInput")
    wd_d = nc.dram_tensor("wd", [EPC, MB, 128, GJ * 128], low,
                          kind="ExternalInput")
    wsg_d = nc.dram_tensor("wsg", [SJ, KB, 128, 256], low,
                           kind="ExternalInput")
    wsd_d = nc.dram_tensor("wsd", [MB, 128, SJ * 128], low,
                           kind="ExternalInput")
    part_d = nc.dram_tensor("part", [H, T], F32, kind="ExternalOutput")

    with TileContext(nc) as tc, ExitStack() as ctx:
        ep = ctx.enter_context  # shorthand

        # ---- resident SBUF ----
        # gate weights first: the router's logits matmul is the head of the
        # PE critical path and must not queue behind the 12MB of x loads.
        cstp = ep(tc.tile_pool(name="cstp", bufs=1))
        wgsb = cstp.tile([128, KB * E], F32, tag="wgsb")
        nc.sync.dma_start(out=wgsb[:, :], in_=wg_d[:, :])
        ident = cstp.tile([128, 128], F32, tag="ident")
        make_identity(nc, ident[:, :])

        xtp = ep(tc.tile_pool(name="xt32p", bufs=KB))
        xt32 = [xtp.tile([128, T], F32, tag="xt32", name=f"xt32_{k}")
                for k in range(KB)]
        if low == F32:
            xlo = xt32
            for k in range(KB):
                nc.sync.dma_start(out=xt32[k][:, :],
                                  in_=xt32_d[ts(k, 128), :])
        else:
            xlp = ep(tc.tile_pool(name="xlop", bufs=KB))
            xlo = [xlp.tile([128, T], low, tag="xlo", name=f"xlo_{k}")
                   for k in range(KB)]
            for k in range(KB):
                nc.sync.dma_start(out=xt32[k][:, :],
                                  in_=xt32_d[ts(k, 128), :])
                nc.sync.dma_start(out=xlo[k][:, :],
                                  in_=xlo_d[ts(k, 128), :])

        actp = ep(tc.tile_pool(name="actp", bufs=EPC * GJ + SJ))
        act = [[actp.tile([128, T], low, tag="act", name=f"act_{e}_{j}")
                for j in range(GJ)] for e in range(EPC)]
        acts = [actp.tile([128, T], low, tag="act", name=f"acts_{j}")
                for j in range(SJ)]
        bcp = ep(tc.tile_pool(name="bcp", bufs=EPC))
        bc = [bcp.tile([128, T], F32, tag="bc", name=f"bc_{e}")
              for e in range(EPC)]

        # gate/up PSUM pools first so they claim banks 0-3; the router's
        # pools live in banks 4-7 and never block the expert matmuls.
        with tc.tile_pool(name="pg_ps", bufs=2, space="PSUM") as pgp, \
             tc.tile_pool(name="pu_ps", bufs=2, space="PSUM") as pup, \
             tc.tile_pool(name="wgb", bufs=6) as wbp, \
             tc.tile_pool(name="silp", bufs=3) as silp:

            # ---- phase 1: router ----
            # logits^T = w_gate^T @ x^T accumulated in one [E, T] psum,
            # transposed back to token-major 128-token tiles on the PE.
            with tc.tile_pool(name="rt_ps", bufs=2, space="PSUM") as rtp, \
                 tc.tile_pool(name="rsm", bufs=3) as rsm, \
                 tc.tile_pool(name="rwk", bufs=3) as rwk:
                with tc.tile_pool(name="lt_ps", bufs=1, space="PSUM") as ltp:
                    lt = ltp.tile([E, T], F32, tag="lt")
                    for k in range(KB):
                        for n in range(2):
                            nc.tensor.matmul(lt[:, ts(n, 512)],
                                             lhsT=wgsb[:, ts(k, E)],
                                             rhs=xt32[k][:, ts(n, 512)],
                                             start=(k == 0),
                                             stop=(k == KB - 1))
                    lts = rsm.tile([E, T], F32, tag="lts")
                    nc.vector.tensor_copy(lts[:, :], lt[:, :])
                with tc.tile_pool(name="ct_ps", bufs=1, space="PSUM") as ctp:
                    ct2 = ctp.tile([EPC, T], F32, tag="ct2")
                    for t in range(TTB):
                        pl = rtp.tile([128, E], F32, tag="pl")
                        nc.tensor.transpose(pl[:, :], lts[:, ts(t, 128)],
                                            ident[0:E, 0:E])
                        nm = rsm.tile([128, 1], F32, tag="nm")
                        nc.vector.tensor_reduce(nm[:, :], pl[:, :], X, A.max,
                                                negate=True)
                        es = rsm.tile([128, E], F32, tag="es")
                        nc.scalar.activation(es[:, :], pl[:, :], AF.Exp,
                                             bias=nm[:, :])
                        gmax = rsm.tile([128, N_GROUP], F32, tag="gmax")
                        nc.vector.tensor_reduce(
                            gmax[:, :],
                            es[:, :].rearrange("p (g e) -> p g e", g=N_GROUP),
                            X, A.max)
                        m1 = rsm.tile([128, 1], F32, tag="m1")
                        nc.vector.tensor_reduce(m1[:, :], gmax[:, :], X, A.max)
                        gz = rsm.tile([128, N_GROUP], F32, tag="gz")
                        nc.vector.scalar_tensor_tensor(
                            out=gz[:, :], in0=gmax[:, :], scalar=m1[:, :],
                            in1=gmax[:, :], op0=A.is_lt, op1=A.mult)
                        m2 = rsm.tile([128, 1], F32, tag="m2")
                        nc.vector.tensor_reduce(m2[:, :], gz[:, :], X, A.max)
                        keep = rsm.tile([128, N_GROUP], F32, tag="keep")
                        nc.vector.tensor_scalar(
                            out=keep[:, :], in0=gmax[:, :], scalar1=m2[:, :],
                            scalar2=None, op0=A.is_ge)
                        msk = rsm.tile([128, E], F32, tag="msk")
                        for g in range(N_GROUP):
                            nc.vector.tensor_scalar(
                                out=msk[:, 4 * g : 4 * g + 4],
                                in0=es[:, 4 * g : 4 * g + 4],
                                scalar1=keep[:, g : g + 1], scalar2=None,
                                op0=A.mult)
                        mxs = rsm.tile([128, TOP_K], F32, tag="mxs")
                        wcur = msk
                        for i in range(TOP_K):
                            nc.vector.tensor_reduce(mxs[:, i : i + 1],
                                                    wcur[:, :], X, A.max)
                            wnxt = rwk.tile([128, E], F32, tag="wk")
                            nc.vector.scalar_tensor_tensor(
                                out=wnxt[:, :], in0=wcur[:, :],
                                scalar=mxs[:, i : i + 1], in1=wcur[:, :],
                                op0=A.is_lt, op1=A.mult)
                            wcur = wnxt
                        wsum = rsm.tile([128, 1], F32, tag="wsum")
                        nc.vector.tensor_reduce(wsum[:, :], mxs[:, :], X,
                                                A.add)
                        rw = rsm.tile([128, 1], F32, tag="rw")
                        nc.vector.reciprocal(rw[:, :], wsum[:, :])
                        sel = rsm.tile([128, E], F32, tag="sel")
                        nc.vector.scalar_tensor_tensor(
                            out=sel[:, :], in0=wcur[:, :], scalar=-1.0,
                            in1=msk[:, :], op0=A.mult, op1=A.add)
                        comb = rsm.tile([128, E], F32, tag="comb")
                        nc.vector.tensor_scalar(
                            out=comb[:, :], in0=sel[:, :], scalar1=rw[:, :],
                            scalar2=float(ROUTED_SCALING), op0=A.mult,
                            op1=A.mult)
                        nc.tensor.transpose(ct2[:, ts(t, 128)],
                                            comb[:, 0:EPC], ident[:, :])
                    rows = rsm.tile([32, T], F32, tag="rows")
                    nc.vector.memset(rows[:, :], 0.0)
                    nc.vector.tensor_copy(rows[0:EPC, :], ct2[:, :])
                rowsb = rsm.tile([32, T], F32, tag="rowsb")
                nc.vector.stream_shuffle(rowsb[:, :], rows[:, :],
                                         mask=[1] + list(range(1, 32)))
                nc.gpsimd.partition_broadcast(bc[0][:, :], rows[0:1, :])
                nc.gpsimd.partition_broadcast(bc[1][:, :], rowsb[0:1, :])

            # ---- phase 2: gate/up matmuls + activations ----
            def gu_pair(w_src, out_tile, bc_tile):
                # w_src: k -> dram AP [128, 256] ([g|u] block)
                pgh = [pgp.tile([128, nsplit], F32, tag="pg",
                                name=f"pg_{h}") for h in range(NH)]
                puh = [pup.tile([128, nsplit], F32, tag="pu",
                                name=f"pu_{h}") for h in range(NH)]
                for k in range(KB):
                    wb = wbp.tile([128, 256], low, tag="wb")
                    nc.sync.dma_start(out=wb[:, :], in_=w_src(k))
                    for h in range(NH):
                        nc.tensor.matmul(pgh[h][:, :], lhsT=wb[:, 0:128],
                                         rhs=xlo[k][:, ts(h, nsplit)],
                                         start=(k == 0), stop=(k == KB - 1))
                    for h in range(NH):
                        nc.tensor.matmul(puh[h][:, :], lhsT=wb[:, 128:256],
                                         rhs=xlo[k][:, ts(h, nsplit)],
                                         start=(k == 0), stop=(k == KB - 1))
                for h in range(NH):
                    hs_ = ts(h, nsplit)
                    sig = silp.tile([128, nsplit], F32, tag="sig")
                    nc.scalar.activation(sig[:, :], pgh[h][:, :], AF.Sigmoid)
                    sil = silp.tile([128, nsplit], F32, tag="sil")
                    nc.vector.scalar_tensor_tensor(
                        out=sil[:, :], in0=pgh[h][:, :], scalar=0.0,
                        in1=sig[:, :], op0=A.bypass, op1=A.mult)
                    if bc_tile is None:
                        nc.vector.scalar_tensor_tensor(
                            out=out_tile[:, hs_], in0=sil[:, :], scalar=0.0,
                            in1=puh[h][:, :], op0=A.bypass, op1=A.mult)
                    else:
                        tmp = silp.tile([128, nsplit], F32, tag="gutmp")
                        nc.vector.scalar_tensor_tensor(
                            out=tmp[:, :], in0=sil[:, :], scalar=0.0,
                            in1=puh[h][:, :], op0=A.bypass, op1=A.mult)
                        nc.vector.scalar_tensor_tensor(
                            out=out_tile[:, hs_], in0=tmp[:, :], scalar=0.0,
                            in1=bc_tile[:, hs_], op0=A.bypass, op1=A.mult)

            for le in range(EPC):
                for j in range(GJ):
                    gu_pair(lambda k, le=le, j=j: wgu_d[le, j, k, :, :],
                            act[le][j], bc[le])
            for j in range(SJ):
                gu_pair(lambda k, j=j: wsg_d[j, k, :, :], acts[j], None)

            # ---- phase 3: down-projection (psum banks 4-7) ----
            with tc.tile_pool(name="dn_ps", bufs=2, space="PSUM") as dnp, \
                 tc.tile_pool(name="wdp", bufs=4) as wdp, \
                 tc.tile_pool(name="wsp", bufs=2) as wsp, \
                 tc.tile_pool(name="outp", bufs=3) as outp:
                n_k = EPC * GJ + SJ
                for m in range(MB):
                    pd = dnp.tile([128, T], F32, tag="pd")
                    slabs = [wdp.tile([128, GJ * 128], low, tag="wdslab",
                                      name=f"wds_{m}_{le}")
                             for le in range(EPC)]
                    for le in range(EPC):
                        nc.sync.dma_start(out=slabs[le][:, :],
                                          in_=wd_d[le, m, :, :])
                    sslab = wsp.tile([128, SJ * 128], low, tag="wsslab")
                    nc.sync.dma_start(out=sslab[:, :], in_=wsd_d[m, :, :])
                    i = 0
                    for le in range(EPC):
                        for j in range(GJ):
                            for n in range(NH):
                                nc.tensor.matmul(
                                    pd[:, ts(n, nsplit)],
                                    lhsT=slabs[le][:, ts(j, 128)],
                                    rhs=act[le][j][:, ts(n, nsplit)],
                                    start=(i == 0), stop=(i == n_k - 1))
                            i += 1
                    for j in range(SJ):
                        for n in range(NH):
                            nc.tensor.matmul(
                                pd[:, ts(n, nsplit)],
                                lhsT=sslab[:, ts(j, 128)],
                                rhs=acts[j][:, ts(n, nsplit)],
                                start=(i == 0), stop=(i == n_k - 1))
                        i += 1
                    osb = outp.tile([128, T], F32, tag="osb")
                    nc.vector.tensor_copy(osb[:, :], pd[:, :])
                    nc.sync.dma_start(out=part_d[ts(m, 128), :],
                                      in_=osb[:, :])

    nc.compile()
    return nc


_CACHE = {}


def _get_nc(low):
    if low not in _CACHE:
        _CACHE[low] = build(low)
    return _CACHE[low]


LOW_DT = mybir.dt.bfloat16
_NP_LO = {F32: np.float32, mybir.dt.bfloat16: ml_dtypes.bfloat16}


def _run(inputs, low=None, trace=False, **kw):
    low = LOW_DT if low is None else low
    nc = _get_nc(low)
    np_lo = _NP_LO[low]
    in_maps = [
        _prep_core(c, inputs["hidden_states"], inputs["w_gate"],
                   inputs["w_gate_up"], inputs["w_down"],
                   inputs["w_shared_gate_up"], inputs["w_shared_down"],
                   np_lo)
        for c in range(N_CORES)
    ]
    res = run_bass_kernel_spmd(nc, in_maps, list(range(N_CORES)),
                               trace=trace, **kw)
    acc = np.zeros((H, T), np.float64)
    for c in range(N_CORES):
        acc += res.results[c]["part"]
    out = np.ascontiguousarray(acc.T).astype(np.float32)
    return out, res


def kernel(**inputs):
    out, _ = _run(inputs)
    return out


# revision 23
# speedup vs baseline: 1.7225x; 1.0654x over previous
"""DeepSeek-style MoE block (grouped top-k routing + 16 routed experts +
shared expert) on 8 Trainium2 NeuronCores.

Sharding: expert-parallel. Core c owns routed experts {2c, 2c+1} (dense
all-token compute, weighted by the combine matrix) plus a 1/8 slice of the
shared expert intermediate dim. Each core computes the full router from a
column-permuted gate matrix so its own experts always land in combine rows
0/1 (keeps the program core-independent). Each core emits an fp32 partial
output [H, T]; partials are summed and transposed on the host.

Math notes:
 - softmax + renormalized top-k weights: the softmax denominator cancels in
   the renormalization, so selection + weights use exp(logit - max) only.
 - ROUTED_SCALING is folded into the combine weights.

All activations/weights are pre-transposed/tiled on the host so every
weight DMA is a single contiguous block and every matmul consumes
[128, 128] stationary slices with [128, T] moving activation slabs.
"""

import sys

sys.path.insert(0, "/opt/trn_rl_repo")

from contextlib import ExitStack

import numpy as np
import ml_dtypes

import concourse.bass as bass
import concourse.mybir as mybir
from concourse import bacc
from concourse.bass import ts
from concourse.tile import TileContext
from concourse.bass_utils import run_bass_kernel_spmd
from concourse.masks import make_identity

F32 = mybir.dt.float32

T, H, E, I = 1024, 2048, 16, 704
IS = 2 * I  # shared expert intermediate
TOP_K, N_GROUP, TOPK_GROUP = 6, 4, 2
ROUTED_SCALING = 2.5

N_CORES = 8
EPC = E // N_CORES  # experts per core (2)
SHI = IS // N_CORES  # shared intermediate slice per core (176)
KB = H // 128  # 16 contraction blocks over hidden dim
GJ = (I + 127) // 128  # 6 col-pair blocks per routed expert
SJ = (SHI + 127) // 128  # 2 col-pair blocks for shared slice
MB = H // 128  # 16 output row blocks
TTB = T // 128  # 8 token tiles


def _expert_perm(c):
    """Permute experts so core c's experts (2c, 2c+1) map to rows 0, 1 while
    preserving the 4-expert group-block structure (group order and
    within-group order are both free)."""
    g = c // 2
    r = (c % 2) * 2
    within = [r, r + 1] + [x for x in range(4) if x not in (r, r + 1)]
    groups = [g] + [x for x in range(N_GROUP) if x != g]
    return [4 * gg + w for gg in groups for w in within]


def _prep_core(c, hs, w_gate, w_gate_up, w_down, w_sgu, w_sd, np_lo):
    f32 = np.float32
    xt = np.ascontiguousarray(np.asarray(hs, f32).T)  # [H, T]
    ins = {"xt32": xt}
    if np_lo != f32:
        ins["xt_lo"] = xt.astype(np_lo)

    perm = _expert_perm(c)
    wg = np.asarray(w_gate, f32)[:, perm]  # [H, E]
    # [128, KB*E]: column k*E+e = w_gate[128k + p, perm[e]]
    ins["wg"] = np.ascontiguousarray(
        wg.reshape(KB, 128, E).transpose(1, 0, 2).reshape(128, KB * E))

    e0 = 2 * c
    wgu = np.asarray(w_gate_up, f32)[e0 : e0 + EPC].astype(np_lo)  # [2,H,2I]
    wdn = np.asarray(w_down, f32)[e0 : e0 + EPC].astype(np_lo)  # [2,I,H]

    # gate/up interleaved blocks: [EPC, GJ, KB, 128, 256] = [g(128) | u(128)]
    wgu_t = np.zeros((EPC, GJ, KB, 128, 256), np_lo)
    # down slabs: [EPC, MB, 128, GJ*128] (row p = concat_j wd[128j+?..] )
    wd_t = np.zeros((EPC, MB, 128, GJ * 128), np_lo)
    for e in range(EPC):
        for j in range(GJ):
            w = min(128, I - 128 * j)
            blk = wgu[e].reshape(KB, 128, 2 * I)
            wgu_t[e, j, :, :, :w] = blk[:, :, 128 * j : 128 * j + w]
            wgu_t[e, j, :, :, 128 : 128 + w] = blk[:, :, I + 128 * j : I + 128 * j + w]
            for m in range(MB):
                wd_t[e, m, :w, 128 * j : 128 * (j + 1)] = \
                    wdn[e, 128 * j : 128 * j + w, 128 * m : 128 * (m + 1)]
    ins["wgu"], ins["wd"] = wgu_t, wd_t

    # shared expert slice: intermediate rows [c*SHI, (c+1)*SHI)
    s0 = c * SHI
    sg = np.asarray(w_sgu, f32)[:, s0 : s0 + SHI].astype(np_lo)
    su = np.asarray(w_sgu, f32)[:, IS + s0 : IS + s0 + SHI].astype(np_lo)
    sd = np.asarray(w_sd, f32)[s0 : s0 + SHI, :].astype(np_lo)

    wsg_t = np.zeros((SJ, KB, 128, 256), np_lo)
    wsd_t = np.zeros((MB, 128, SJ * 128), np_lo)
    for j in range(SJ):
        w = min(128, SHI - 128 * j)
        wsg_t[j, :, :, :w] = sg.reshape(KB, 128, SHI)[:, :, 128 * j : 128 * j + w]
        wsg_t[j, :, :, 128 : 128 + w] = \
            su.reshape(KB, 128, SHI)[:, :, 128 * j : 128 * j + w]
        for m in range(MB):
            wsd_t[m, :w, 128 * j : 128 * (j + 1)] = \
                sd[128 * j : 128 * j + w, 128 * m : 128 * (m + 1)]
    ins["wsg"], ins["wsd"] = wsg_t, wsd_t
    return ins


def build(low=F32, nsplit=None):
    nc = bacc.Bacc("TRN2", target_bir_lowering=False, debug=False,
                   num_devices=N_CORES)
    A = mybir.AluOpType
    X = mybir.AxisListType.X
    AF = mybir.ActivationFunctionType
    # matmul output must stay within one 2KB PSUM bank -> N <= 512 fp32
    if nsplit is None:
        nsplit = 512
    NH = T // nsplit

    xt32_d = nc.dram_tensor("xt32", [H, T], F32, kind="ExternalInput")
    xlo_d = (xt32_d if low == F32 else
             nc.dram_tensor("xt_lo", [H, T], low, kind="ExternalInput"))
    wg_d = nc.dram_tensor("wg", [128, KB * E], F32, kind="ExternalInput")
    wgu_d = nc.dram_tensor("wgu", [EPC, GJ, KB, 128, 256], low,
                           kind="ExternalInput")
    wd_d = nc.dram_tensor("wd", [EPC, MB, 128, GJ * 128], low,
                          kind="ExternalInput")
    wsg_d = nc.dram_tensor("wsg", [SJ, KB, 128, 256], low,
                           kind="ExternalInput")
    wsd_d = nc.dram_tensor("wsd", [MB, 128, SJ * 128], low,
                           kind="ExternalInput")
    part_d = nc.dram_tensor("part", [H, T], F32, kind="ExternalOutput")

    with TileContext(nc) as tc, ExitStack() as ctx:
        ep = ctx.enter_context  # shorthand

        # ---- resident SBUF ----
        # gate weights first: the router's logits matmul is the head of the
        # PE critical path and must not queue behind the 12MB of x loads.
        cstp = ep(tc.tile_pool(name="cstp", bufs=1))
        wgsb = cstp.tile([128, KB * E], F32, tag="wgsb")
        # k=0 block first: it alone gates the very first router matmul
        nc.sync.dma_start(out=wgsb[:, 0:E], in_=wg_d[:, 0:E])
        nc.sync.dma_start(out=wgsb[:, E:], in_=wg_d[:, E:])
        ident = cstp.tile([128, 128], F32, tag="ident")
        make_identity(nc, ident[:, :])

        xtp = ep(tc.tile_pool(name="xt32p", bufs=KB))
        xt32 = [xtp.tile([128, T], F32, tag="xt32", name=f"xt32_{k}")
                for k in range(KB)]
        for k in range(KB):
            nc.sync.dma_start(out=xt32[k][:, :], in_=xt32_d[ts(k, 128), :])
        if low == F32:
            xlo = xt32
        else:
            # xlo DMAs are emitted inside the first gate/up pair so their
            # issue slots interleave with that pair's weight blocks instead
            # of delaying them behind 4MB of bulk loads.
            xlp = ep(tc.tile_pool(name="xlop", bufs=KB))
            xlo = [xlp.tile([128, T], low, tag="xlo", name=f"xlo_{k}")
                   for k in range(KB)]

        actp = ep(tc.tile_pool(name="actp", bufs=EPC * GJ + SJ))
        act = [[actp.tile([128, T], low, tag="act", name=f"act_{e}_{j}")
                for j in range(GJ)] for e in range(EPC)]
        acts = [actp.tile([128, T], low, tag="act", name=f"acts_{j}")
                for j in range(SJ)]
        bcp = ep(tc.tile_pool(name="bcp", bufs=EPC))
        bc = [bcp.tile([128, T], F32, tag="bc", name=f"bc_{e}")
              for e in range(EPC)]

        # gate/up PSUM pools first so they claim banks 0-3; the router's
        # pools live in banks 4-7 and never block the expert matmuls.
        with tc.tile_pool(name="pg_ps", bufs=2, space="PSUM") as pgp, \
             tc.tile_pool(name="pu_ps", bufs=2, space="PSUM") as pup, \
             tc.tile_pool(name="wgb", bufs=10) as wbp, \
             tc.tile_pool(name="silp", bufs=3) as silp:

            # ---- phase 1: router ----
            # logits^T = w_gate^T @ x^T accumulated in one [E, T] psum,
            # transposed back to token-major 128-token tiles on the PE.
            with tc.tile_pool(name="rt_ps", bufs=2, space="PSUM") as rtp, \
                 tc.tile_pool(name="rsm", bufs=3) as rsm, \
                 tc.tile_pool(name="rwk", bufs=3) as rwk:
                with tc.tile_pool(name="lt_ps", bufs=1, space="PSUM") as ltp:
                    lt = ltp.tile([E, T], F32, tag="lt")
                    for k in range(KB):
                        for n in range(2):
                            nc.tensor.matmul(lt[:, ts(n, 512)],
                                             lhsT=wgsb[:, ts(k, E)],
                                             rhs=xt32[k][:, ts(n, 512)],
                                             start=(k == 0),
                                             stop=(k == KB - 1))
                    lts = rsm.tile([E, T], F32, tag="lts")
                    nc.vector.tensor_copy(lts[:, :], lt[:, :])
                with tc.tile_pool(name="ct_ps", bufs=1, space="PSUM") as ctp:
                    ct2 = ctp.tile([EPC, T], F32, tag="ct2")
                    for t in range(TTB):
                        pl = rtp.tile([128, E], F32, tag="pl")
                        nc.tensor.transpose(pl[:, :], lts[:, ts(t, 128)],
                                            ident[0:E, 0:E])
                        nm = rsm.tile([128, 1], F32, tag="nm")
                        nc.vector.tensor_reduce(nm[:, :], pl[:, :], X, A.max,
                                                negate=True)
                        es = rsm.tile([128, E], F32, tag="es")
                        nc.scalar.activation(es[:, :], pl[:, :], AF.Exp,
                                             bias=nm[:, :])
                        gmax = rsm.tile([128, N_GROUP], F32, tag="gmax")
                        nc.vector.tensor_reduce(
                            gmax[:, :],
                            es[:, :].rearrange("p (g e) -> p g e", g=N_GROUP),
                            X, A.max)
                        m1 = rsm.tile([128, 1], F32, tag="m1")
                        nc.vector.tensor_reduce(m1[:, :], gmax[:, :], X, A.max)
                        gz = rsm.tile([128, N_GROUP], F32, tag="gz")
                        nc.vector.scalar_tensor_tensor(
                            out=gz[:, :], in0=gmax[:, :], scalar=m1[:, :],
                            in1=gmax[:, :], op0=A.is_lt, op1=A.mult)
                        m2 = rsm.tile([128, 1], F32, tag="m2")
                        nc.vector.tensor_reduce(m2[:, :], gz[:, :], X, A.max)
                        keep = rsm.tile([128, N_GROUP], F32, tag="keep")
                        nc.vector.tensor_scalar(
                            out=keep[:, :], in0=gmax[:, :], scalar1=m2[:, :],
                            scalar2=None, op0=A.is_ge)
                        msk = rsm.tile([128, E], F32, tag="msk")
                        for g in range(N_GROUP):
                            nc.vector.tensor_scalar(
                                out=msk[:, 4 * g : 4 * g + 4],
                                in0=es[:, 4 * g : 4 * g + 4],
                                scalar1=keep[:, g : g + 1], scalar2=None,
                                op0=A.mult)
                        mxs = rsm.tile([128, TOP_K], F32, tag="mxs")
                        wcur = msk
                        for i in range(TOP_K):
                            nc.vector.tensor_reduce(mxs[:, i : i + 1],
                                                    wcur[:, :], X, A.max)
                            wnxt = rwk.tile([128, E], F32, tag="wk")
                            nc.vector.scalar_tensor_tensor(
                                out=wnxt[:, :], in0=wcur[:, :],
                                scalar=mxs[:, i : i + 1], in1=wcur[:, :],
                                op0=A.is_lt, op1=A.mult)
                            wcur = wnxt
                        wsum = rsm.tile([128, 1], F32, tag="wsum")
                        nc.vector.tensor_reduce(wsum[:, :], mxs[:, :], X,
                                                A.add)
                        rw = rsm.tile([128, 1], F32, tag="rw")
                        nc.vector.reciprocal(rw[:, :], wsum[:, :])
                        sel = rsm.tile([128, E], F32, tag="sel")
                        nc.vector.scalar_tensor_tensor(
                            out=sel[:, :], in0=wcur[:, :], scalar=-1.0,
                            in1=msk[:, :], op0=A.mult, op1=A.add)
                        comb = rsm.tile([128, E], F32, tag="comb")
                        nc.vector.tensor_scalar(
                            out=comb[:, :], in0=sel[:, :], scalar1=rw[:, :],
                            scalar2=float(ROUTED_SCALING), op0=A.mult,
                            op1=A.mult)
                        nc.tensor.transpose(ct2[:, ts(t, 128)],
                                            comb[:, 0:EPC], ident[:, :])
                    rows = rsm.tile([32, T], F32, tag="rows")
                    nc.vector.memset(rows[:, :], 0.0)
                    nc.vector.tensor_copy(rows[0:EPC, :], ct2[:, :])
                rowsb = rsm.tile([32, T], F32, tag="rowsb")
                nc.vector.stream_shuffle(rowsb[:, :], rows[:, :],
                                         mask=[1] + list(range(1, 32)))
                nc.gpsimd.partition_broadcast(bc[0][:, :], rows[0:1, :])
                nc.gpsimd.partition_broadcast(bc[1][:, :], rowsb[0:1, :])

            # ---- phase 2: gate/up matmuls + activations ----
            def gu_pair(w_src, out_tile, bc_tile, load_xlo=False):
                # w_src: k -> dram AP [128, 256] ([g|u] block)
                pgh = [pgp.tile([128, nsplit], F32, tag="pg",
                                name=f"pg_{h}") for h in range(NH)]
                puh = [pup.tile([128, nsplit], F32, tag="pu",
                                name=f"pu_{h}") for h in range(NH)]
                for k in range(KB):
                    if load_xlo:
                        nc.sync.dma_start(out=xlo[k][:, :],
                                          in_=xlo_d[ts(k, 128), :])
                    wb = wbp.tile([128, 256], low, tag="wb")
                    nc.sync.dma_start(out=wb[:, :], in_=w_src(k))
                    for h in range(NH):
                        nc.tensor.matmul(pgh[h][:, :], lhsT=wb[:, 0:128],
                                         rhs=xlo[k][:, ts(h, nsplit)],
                                         start=(k == 0), stop=(k == KB - 1))
                    for h in range(NH):
                        nc.tensor.matmul(puh[h][:, :], lhsT=wb[:, 128:256],
                                         rhs=xlo[k][:, ts(h, nsplit)],
                                         start=(k == 0), stop=(k == KB - 1))
                for h in range(NH):
                    hs_ = ts(h, nsplit)
                    sig = silp.tile([128, nsplit], F32, tag="sig")
                    nc.scalar.activation(sig[:, :], pgh[h][:, :], AF.Sigmoid)
                    sil = silp.tile([128, nsplit], F32, tag="sil")
                    nc.vector.scalar_tensor_tensor(
                        out=sil[:, :], in0=pgh[h][:, :], scalar=0.0,
                        in1=sig[:, :], op0=A.bypass, op1=A.mult)
                    if bc_tile is None:
                        nc.vector.scalar_tensor_tensor(
                            out=out_tile[:, hs_], in0=sil[:, :], scalar=0.0,
                            in1=puh[h][:, :], op0=A.bypass, op1=A.mult)
                    else:
                        tmp = silp.tile([128, nsplit], F32, tag="gutmp")
                        nc.vector.scalar_tensor_tensor(
                            out=tmp[:, :], in0=sil[:, :], scalar=0.0,
                            in1=puh[h][:, :], op0=A.bypass, op1=A.mult)
                        nc.vector.scalar_tensor_tensor(
                            out=out_tile[:, hs_], in0=tmp[:, :], scalar=0.0,
                            in1=bc_tile[:, hs_], op0=A.bypass, op1=A.mult)

            first = low != F32
            for le in range(EPC):
                for j in range(GJ):
                    gu_pair(lambda k, le=le, j=j: wgu_d[le, j, k, :, :],
                            act[le][j], bc[le], load_xlo=first)
                    first = False
            for j in range(SJ):
                gu_pair(lambda k, j=j: wsg_d[j, k, :, :], acts[j], None)

            # ---- phase 3: down-projection (psum banks 4-7) ----
            with tc.tile_pool(name="dn_ps", bufs=2, space="PSUM") as dnp, \
                 tc.tile_pool(name="wdp", bufs=4) as wdp, \
                 tc.tile_pool(name="wsp", bufs=2) as wsp, \
                 tc.tile_pool(name="outp", bufs=3) as outp:
                n_k = EPC * GJ + SJ
                for m in range(MB):
                    pd = dnp.tile([128, T], F32, tag="pd")
                    slabs = [wdp.tile([128, GJ * 128], low, tag="wdslab",
                                      name=f"wds_{m}_{le}")
                             for le in range(EPC)]
                    for le in range(EPC):
                        nc.sync.dma_start(out=slabs[le][:, :],
                                          in_=wd_d[le, m, :, :])
                    sslab = wsp.tile([128, SJ * 128], low, tag="wsslab")
                    nc.sync.dma_start(out=sslab[:, :], in_=wsd_d[m, :, :])
                    i = 0
                    for le in range(EPC):
                        for j in range(GJ):
                            for n in range(NH):
                                nc.tensor.matmul(
                                    pd[:, ts(n, nsplit)],
                                    lhsT=slabs[le][:, ts(j, 128)],
                                    rhs=act[le][j][:, ts(n, nsplit)],
                                    start=(i == 0), stop=(i == n_k - 1))
                            i += 1
                    for j in range(SJ):
                        for n in range(NH):
                            nc.tensor.matmul(
                                pd[:, ts(n, nsplit)],
                                lhsT=sslab[:, ts(j, 128)],
                                rhs=acts[j][:, ts(n, nsplit)],
                                start=(i == 0), stop=(i == n_k - 1))
                        i += 1
                    osb = outp.tile([128, T], F32, tag="osb")
                    nc.vector.tensor_copy(osb[:, :], pd[:, :])
                    nc.sync.dma_start(out=part_d[ts(m, 128), :],
                                      in_=osb[:, :])

    nc.compile()
    return nc


_CACHE = {}


def _get_nc(low):
    if low not in _CACHE:
        _CACHE[low] = build(low)
    return _CACHE[low]


LOW_DT = mybir.dt.bfloat16
_NP_LO = {F32: np.float32, mybir.dt.bfloat16: ml_dtypes.bfloat16}


def _run(inputs, low=None, trace=False, **kw):
    low = LOW_DT if low is None else low
    nc = _get_nc(low)
    np_lo = _NP_LO[low]
    in_maps = [
        _prep_core(c, inputs["hidden_states"], inputs["w_gate"],
                   inputs["w_gate_up"], inputs["w_down"],
                   inputs["w_shared_gate_up"], inputs["w_shared_down"],
                   np_lo)
        for c in range(N_CORES)
    ]
    res = run_bass_kernel_spmd(nc, in_maps, list(range(N_CORES)),
                               trace=trace, **kw)
    acc = np.zeros((H, T), np.float64)
    for c in range(N_CORES):
        acc += res.results[c]["part"]
    out = np.ascontiguousarray(acc.T).astype(np.float32)
    return out, res


def kernel(**inputs):
    out, _ = _run(inputs)
    return out
